# revision 3
# baseline (speedup 1.0000x reference)
import numpy as np
import jax
import jax.numpy as jnp

# nn_HWTConv2D: B=16, C=64, H=W=256, P=2 pods. Data-parallel over batch on 8 cores.
B, C, H, W, P = 16, 64, 256, 256, 2
NCORES = 8
NORM = float(1.0 / np.sqrt(2.0))


def _haar_matrix(n):
    # Orthonormal multilevel 1D Haar matrix: haar1d_fwd(x) == Hm @ x.
    m = int(np.log2(n))
    Hm = np.eye(n, dtype=np.float64)
    length = n
    for _ in range(m):
        L = np.eye(n, dtype=np.float64)
        half = length // 2
        blk = np.zeros((length, length), dtype=np.float64)
        for i in range(half):
            blk[i, 2 * i] = NORM
            blk[i, 2 * i + 1] = NORM
            blk[half + i, 2 * i] = NORM
            blk[half + i, 2 * i + 1] = -NORM
        L[:length, :length] = blk
        Hm = L @ Hm
        length //= 2
    return Hm.astype(np.float32)


_HM = _haar_matrix(H)  # (256, 256), orthonormal: inverse = HM.T


def _shard_fn(x, v, conv_w, tau, hm, hmT):
    # x: (B/8, C, H, W). F = hm @ X @ hmT applied per (b, c) plane.
    hp = jax.lax.Precision.HIGHEST
    f1 = jnp.matmul(jnp.matmul(hm, x, precision=hp), hmT, precision=hp)
    acc = f1
    for i in range(P):
        f3 = (f1 * v[i]).reshape(x.shape[0], C, H * W)
        f4 = jnp.matmul(conv_w[i], f3, precision=hp).reshape(x.shape)
        f5 = f4 - jnp.clip(f4, -tau[i], tau[i])
        acc = acc + f5
    # residual folded in wavelet domain (acc started from f1): y = hmT(acc)hm
    return jnp.matmul(jnp.matmul(hmT, acc, precision=hp), hm, precision=hp)


_jitted = jax.jit(_shard_fn)


def kernel(x, v, conv_w, tau):
    devs = jax.devices()[:NCORES]
    xs = x.reshape(NCORES, B // NCORES, C, H, W)
    hmT = np.ascontiguousarray(_HM.T)
    outs = []
    for d in range(NCORES):
        args = [jax.device_put(a, devs[d]) for a in (xs[d], v, conv_w, tau, _HM, hmT)]
        outs.append(_jitted(*args))
    y = np.concatenate([np.asarray(o) for o in outs], axis=0)
    return y.reshape(B, C, H, W).astype(np.float32)
